# revision 26
# baseline (speedup 1.0000x reference)
"""Trainium2 Bass kernel for nn_AutoregressiveDecoder (8-core data parallel).

Strategy:
  - Pure data parallel: B=16384 rows sharded 2048/core across 8 NeuronCores.
  - MLP compute runs feature-major (features on partitions, batch on the free
    dim) so weights act as the matmul stationary operand.
  - The big matmuls (seq@w1, h1@w2, h2@w3) run as fp8-e4m3 DoubleRow
    (K=256/instruction). Weights are host-scaled by 64 so fp8 sees
    ~unit-variance values; the 1/64 descale folds into the consumer ACT
    `scale` (or the post-L3 affine). h1/h2 activations are written fp8 by
    the gelu itself; bases and state stay bf16/f32.
  - seq_embed @ w1[:512] is step-invariant -> computed once per 512-row
    macro-tile ("base", bf16), per-step the small state/onehot extras
    (K=9/15, quad-packed into all four PE row-groups) are matmul'd and the
    base re-added via a bf16 identity matmul in the same PSUM group.
  - L3 merges both heads into one zero-padded [32, NB] PSUM accumulation
    (logit col 0, freq/enrich cols 1:2) -> single StreamTranspose back, with
    descale+bias fused into the tensor_scalar that writes the raw-output
    tiles (also the host-side loss inputs).
  - Per-row scalar plumbing (sigmoid via tanh, clips, selects, state
    scatter) runs in a blocked batch-major layout [32 partitions,
    16 blocks x 32 slots], bridged with 32x32 StreamTransposes on DVE.
  - Index-only preprocessing (ALL_PERMS lookup, one-hot, take_along_axis
    gathers) happens host-side in numpy; loss sums are finished host-side
    from the raw logits/preds (psum of scalars x 8 cores).
"""

import numpy as np
import ml_dtypes

import concourse.bass as bass
import concourse.bacc as bacc
import concourse.tile as tile
from concourse import mybir
from concourse.bass_utils import run_bass_kernel_spmd

BF16 = mybir.dt.bfloat16
F8 = mybir.dt.float8e4
F32 = mybir.dt.float32
AF = mybir.ActivationFunctionType
ALU = mybir.AluOpType
DRMODE = mybir.MatmulPerfMode.DoubleRow
NP_BF16 = ml_dtypes.bfloat16
NP_F8 = ml_dtypes.float8_e4m3
SC8 = 64.0
ISC8 = 1.0 / SC8

B, D, H = 16384, 512, 512
NCORES = 8
NB = 512            # macro-tile rows (matmul free dim)
ALL_PERMS = np.array(
    [[0, 1, 2], [0, 2, 1], [1, 0, 2], [1, 2, 0], [2, 0, 1], [2, 1, 0]], np.int32
)

# blocked-layout slot map (32 slots per 32-row block), r-major:
# slot 3r+0 = pres_r, 3r+1 = freq_r, 3r+2 = enrich_r; 9:12 flags; 12:15 roh
S_FL, S_ROH = 9, 12


def r3(t, s):
    """view a [32, 16*s] tile as [32 p, 16 j, s slots]"""
    return t[:, :].rearrange("p (j s) -> p j s", s=s)


def _enable_ldw_opt():
    """walrus --enable-ldw-opt=false is hardcoded; flip it (dedups LDWEIGHTS)."""
    from concourse import bass_utils as bu
    if getattr(bu, "_ldw_patched", False):
        return
    orig = bu.run_command

    def patched(cmd, *a, **k):
        cmd = list(cmd)  # ldw-opt=true crashes walrus on this BIR; keep off
        return orig(cmd, *a, **k)

    bu.run_command = patched
    bu._ldw_patched = True


def build_graph(BL):
    """Build the per-core Bass graph. BL = rows per core (multiple of NB)."""
    _enable_ldw_opt()
    NM = BL // NB          # macro-tiles per core
    NBLK = NB // 32        # 32-row blocks per macro-tile (16)
    BLKT = BL // 32        # total blocks per core

    nc = bacc.Bacc("TRN2", target_bir_lowering=False, debug=False,
                   num_devices=NCORES)

    # ---- dram parameters -------------------------------------------------
    U8 = mybir.dt.uint8
    seq_d = nc.dram_tensor("seq", [D, BL], F8, kind="ExternalInput").ap()
    gts_d = nc.dram_tensor("gts", [96, BLKT * 3], F32, kind="ExternalInput").ap()
    roh_d = nc.dram_tensor("roh", [96, BLKT * 3], F32, kind="ExternalInput").ap()
    # uint8 copies of the masks (CopyPredicated wants integer predicates)
    mski_d = nc.dram_tensor("mski", [96, BLKT * 3], U8, kind="ExternalInput").ap()
    roi9_d = nc.dram_tensor("roi9", [96, BLKT * 9], U8, kind="ExternalInput").ap()

    pw1_d = nc.dram_tensor("pw1", [D, H], F8, kind="ExternalInput").ap()
    w1xq_d = nc.dram_tensor("w1xq", [128, H], BF16, kind="ExternalInput").ap()
    pb1_d = nc.dram_tensor("pb1", [H], F32, kind="ExternalInput").ap()
    pw2_d = nc.dram_tensor("pw2", [H, H // 2], F8, kind="ExternalInput").ap()
    pb2_d = nc.dram_tensor("pb2", [H // 2], F32, kind="ExternalInput").ap()
    pw3_d = nc.dram_tensor("pw3p", [H // 2, 32], F8, kind="ExternalInput").ap()

    fw1_d = nc.dram_tensor("fw1", [D, H], F8, kind="ExternalInput").ap()
    fb1_d = nc.dram_tensor("fb1", [H], F32, kind="ExternalInput").ap()
    fw2_d = nc.dram_tensor("fw2", [H, H], F8, kind="ExternalInput").ap()
    fb2_d = nc.dram_tensor("fb2", [H], F32, kind="ExternalInput").ap()
    fw3_d = nc.dram_tensor("fw3p", [H, 32], F8, kind="ExternalInput").ap()
    eye_d = nc.dram_tensor("eye", [128, 128], BF16, kind="ExternalInput").ap()
    b3s_d = nc.dram_tensor("b3s", [1, 3], F32, kind="ExternalInput").ap()

    dpfe_d = nc.dram_tensor("dpfe", [BL, 9], F32, kind="ExternalOutput").ap()
    # raw per-step MLP outputs (loss terms are finished host-side)
    raw9_d = nc.dram_tensor("raw9", [BL, 9], F32, kind="ExternalOutput").ap()

    v = nc.vector
    sc = nc.scalar
    te = nc.tensor

    with tile.TileContext(nc) as tc:
        wpool = tc.alloc_tile_pool(name="w", bufs=1)
        pers = tc.alloc_tile_pool(name="pers", bufs=1)
        bigp = tc.alloc_tile_pool(name="big", bufs=3)
        stp = tc.alloc_tile_pool(name="stp", bufs=2)
        smp = tc.alloc_tile_pool(name="smp", bufs=4)
        psp = tc.alloc_tile_pool(name="psum", bufs=6, space="PSUM")
        ps3 = tc.alloc_tile_pool(name="psum3", bufs=2, space="PSUM")

        # ---- load weights (persistent) ----------------------------------
        pw1_sb = wpool.tile([128, 4 * H], F8)
        fw1_sb = wpool.tile([128, 4 * H], F8)
        pw2_sb = wpool.tile([128, 4 * (H // 2)], F8)
        fw2_sb = wpool.tile([128, 4 * H], F8)
        nc.gpsimd.dma_start(
            pw1_sb[:, :].rearrange("p (k m) -> p k m", k=4),
            pw1_d[:, :].rearrange("(k p) m -> p k m", k=4))
        nc.gpsimd.dma_start(
            fw1_sb[:, :].rearrange("p (k m) -> p k m", k=4),
            fw1_d[:, :].rearrange("(k p) m -> p k m", k=4))
        # biases as [128, nchunk] f32 (partition p, chunk m) for ACT bias APs
        pb1_sb = wpool.tile([128, 4], F32)
        fb1_sb = wpool.tile([128, 4], F32)
        fb2_sb = wpool.tile([128, 4], F32)
        pb2_sb = wpool.tile([128, 2], F32)
        nc.scalar.dma_start(pb1_sb[:, :],
                            pb1_d.rearrange("(m p) -> p m", p=128))
        nc.scalar.dma_start(fb1_sb[:, :],
                            fb1_d.rearrange("(m p) -> p m", p=128))
        for k in range(4):
            nc.gpsimd.dma_start(pw2_sb[:, k * 256:(k + 1) * 256],
                                pw2_d[k * 128:(k + 1) * 128, :])
            nc.gpsimd.dma_start(fw2_sb[:, k * H:(k + 1) * H],
                                fw2_d[k * 128:(k + 1) * 128, :])
        ones3 = wpool.tile([32, NBLK * 3], F32)
        v.memset(ones3[:, :], 1.0)

        # PE warm-up: ~100 tiny matmuls on a zeroed tile during the DMA
        # preamble so phase 1 starts at the full 2.4 GHz HAM clock
        warm = wpool.tile([128, 128], BF16)
        v.memset(warm[:, :], 0.0)
        pwarm = psp.tile([128, NB], F32, tag="ps_main")
        for _ in range(40):
            te.matmul(pwarm[:, 0:128], warm[:, :], warm[:, :],
                      start=True, stop=True)

        # ---- phase 1: all macro-tile bases (PE-dense, keeps HAM warm) ---
        base_p, base_f, sts, raw9 = {}, {}, {}, {}
        pw13 = pw1_sb[:, :].rearrange("p (k m) -> p k m", k=4)
        fw13 = fw1_sb[:, :].rearrange("p (k m) -> p k m", k=4)
        for mt in range(NM):
            rows = slice(mt * NB, (mt + 1) * NB)
            seqT = stp.tile([128, 4 * NB], F8, tag="seqT", name=f"seqT{mt}")
            sq3 = seqT[:, :].rearrange("p (k n) -> p k n", k=4)
            nc.sync.dma_start(
                sq3[:, :, :],
                seq_d[:, rows].rearrange("(k p) n -> p k n", k=4))
            base_p[mt] = pers.tile([128, 4 * NB], BF16, tag=f"base_p{mt}", name=f"base_p{mt}")
            base_f[mt] = pers.tile([128, 4 * NB], BF16, tag=f"base_f{mt}", name=f"base_f{mt}")
            for m in range(4):
                pp = psp.tile([128, NB], F32, tag="ps_main")
                for j in range(2):
                    te.matmul(pp[:, :],
                              pw13[:, 2 * j:2 * j + 2, m * 128:(m + 1) * 128],
                              sq3[:, 2 * j:2 * j + 2, :],
                              start=(j == 0), stop=(j == 1),
                              perf_mode=DRMODE)
                v.tensor_scalar(base_p[mt][:, m * NB:(m + 1) * NB], pp[:, :],
                                ISC8, pb1_sb[:, m:m + 1], ALU.mult, ALU.add)
                pf_ = psp.tile([128, NB], F32, tag="ps_main")
                for j in range(2):
                    te.matmul(pf_[:, :],
                              fw13[:, 2 * j:2 * j + 2, m * 128:(m + 1) * 128],
                              sq3[:, 2 * j:2 * j + 2, :],
                              start=(j == 0), stop=(j == 1),
                              perf_mode=DRMODE)
                v.tensor_scalar(base_f[mt][:, m * NB:(m + 1) * NB], pf_[:, :],
                                ISC8, fb1_sb[:, m:m + 1], ALU.mult, ALU.add)
            sts[mt] = pers.tile([32, NBLK * 32], F32, tag=f"st{mt}", name=f"st{mt}")
            v.memset(sts[mt][:, :], 0.0)
            raw9[mt] = pers.tile([32, NBLK * 9], F32, tag=f"r9{mt}",
                                 name=f"r9{mt}")

        nc.sync.dma_start(fb2_sb[:, :],
                            fb2_d.rearrange("(m p) -> p m", p=128))
        nc.sync.dma_start(pb2_sb[:, :],
                            pb2_d.rearrange("(m p) -> p m", p=128))
        w1xq_sb = wpool.tile([128, H], BF16)
        nc.sync.dma_start(w1xq_sb[:, :], w1xq_d[:, :])
        pw3_sb = wpool.tile([128, 64], F8)   # 2 k-chunks x 32 cols
        fw3_sb = wpool.tile([128, 128], F8)  # 4 k-chunks x 32 cols
        for k in range(2):
            nc.sync.dma_start(pw3_sb[:, k * 32:(k + 1) * 32],
                                pw3_d[k * 128:(k + 1) * 128, :])
        for k in range(4):
            nc.sync.dma_start(fw3_sb[:, k * 32:(k + 1) * 32],
                                fw3_d[k * 128:(k + 1) * 128, :])
        eye_sb = wpool.tile([128, 128], BF16)
        nc.sync.dma_start(eye_sb[:, :], eye_d[:, :])
        # l3 output biases broadcast to 32 partitions: cols (pb3, fb3_0, fb3_1)
        b3s_sb = wpool.tile([1, 3], F32)
        nc.sync.dma_start(b3s_sb[:, :], b3s_d[:, :])
        b3bc = wpool.tile([32, 3], F32)
        nc.gpsimd.partition_broadcast(b3bc[:, :], b3s_sb[:, :])

        # preload all per-step inputs (tiny; frees the sync queue and
        # removes DMA latency from every step start)
        gts_sb, mks_sb, rois_sb, rohs_sb = {}, {}, {}, {}
        for s in range(3):
            rs = slice(s * 32, (s + 1) * 32)
            gts_sb[s] = pers.tile([32, BLKT * 3], F32, tag=f"gts{s}",
                                  name=f"gts{s}")
            nc.sync.dma_start(gts_sb[s][:, :], gts_d[rs, :])
            mks_sb[s] = pers.tile([32, BLKT * 3], U8, tag=f"mks{s}",
                                  name=f"mks{s}")
            nc.sync.dma_start(mks_sb[s][:, :], mski_d[rs, :])
            rois_sb[s] = pers.tile([32, BLKT * 9], U8, tag=f"rois{s}",
                                   name=f"rois{s}")
            nc.sync.dma_start(rois_sb[s][:, :], roi9_d[rs, :])
            rohs_sb[s] = pers.tile([32, BLKT * 3], F32, tag=f"rohs{s}",
                                   name=f"rohs{s}")
            nc.sync.dma_start(rohs_sb[s][:, :], roh_d[rs, :])

        # ---- phase 2: autoregressive steps, macro-interleaved -----------
        # While macro mt's plumbing runs on DVE/ACT, the PE executes the
        # other macros' matmuls (engines are in-order; interleaved emission
        # is what lets the scheduler fill the gaps).
        for s in range(3):
            for mt in range(NM):
                st3 = r3(sts[mt], 32)
                c3 = slice(mt * NBLK * 3, (mt + 1) * NBLK * 3)
                c9 = slice(mt * NBLK * 9, (mt + 1) * NBLK * 9)
                gt3 = gts_sb[s][:, c3].rearrange("p (j s) -> p j s", s=3)
                mk3 = mks_sb[s][:, c3].rearrange("p (j s) -> p j s", s=3)
                roi9 = rois_sb[s][:, c9].rearrange("p (j s) -> p j s", s=9)
                roh3 = rohs_sb[s][:, c3].rearrange("p (j s) -> p j s", s=3)

                # bridge to feature-major: cast + stream-transpose
                # (st slots 6..8 stay zero; the one-hot joins here)
                st_bf = smp.tile([32, NBLK * 32], BF16, tag="stbf")
                v.tensor_copy(st_bf[:, :], sts[mt][:, :])
                v.tensor_copy(r3(st_bf, 32)[:, :, S_ROH:S_ROH + 3],
                              roh3[:, :, :])  # slots 12:15
                exT = smp.tile([128, NBLK * 32], BF16, tag="exT")
                v.transpose(exT[0:32, :], st_bf[:, :])
                v.tensor_copy(exT[32:47, :], exT[0:15, :])
                v.tensor_copy(exT[64:79, :], exT[0:15, :])
                v.tensor_copy(exT[96:111, :], exT[0:15, :])

                # layer 1: base + extra @ w1x -> gelu -> h1 (bf16)
                h1p = bigp.tile([128, 4 * NB], F8, tag="h1p")
                h1f = bigp.tile([128, 4 * NB], F8, tag="h1f")
                for half in range(2):
                    quad = []
                    for i in range(2):
                        m = 2 * half + i
                        pp = psp.tile([128, NB], F32, tag="ps_main")
                        pf_ = psp.tile([128, NB], F32, tag="ps_main")
                        te.matmul(pp[:, :],
                                  w1xq_sb[32 * i:32 * i + 15,
                                          m * 128:(m + 1) * 128],
                                  exT[32 * i:32 * i + 15, :],
                                  start=True, stop=False,
                                  tile_position=(32 * i, 0))
                        te.matmul(pf_[:, :],
                                  w1xq_sb[64 + 32 * i:64 + 32 * i + 15,
                                          m * 128:(m + 1) * 128],
                                  exT[64 + 32 * i:64 + 32 * i + 15, :],
                                  start=True, stop=False,
                                  tile_position=(64 + 32 * i, 0))
                        quad.append((m, pp, pf_))
                    for m, pp, pf_ in quad:
                        te.matmul(pp[:, :], eye_sb[:, :],
                                  base_p[mt][:, m * NB:(m + 1) * NB],
                                  start=False, stop=True)
                        te.matmul(pf_[:, :], eye_sb[:, :],
                                  base_f[mt][:, m * NB:(m + 1) * NB],
                                  start=False, stop=True)
                        sc.activation(h1p[:, m * NB:(m + 1) * NB], pp[:, :],
                                      AF.Gelu)
                        sc.activation(h1f[:, m * NB:(m + 1) * NB], pf_[:, :],
                                      AF.Gelu)

                # layer 2
                h2p = bigp.tile([128, 2 * NB], F8, tag="h2p")
                h1p3 = h1p[:, :].rearrange("p (k n) -> p k n", k=4)
                pw23 = pw2_sb[:, :].rearrange("p (k m) -> p k m", k=4)
                for m in range(2):
                    pp = psp.tile([128, NB], F32, tag="ps_main")
                    for j in range(2):
                        te.matmul(pp[:, :],
                                  pw23[:, 2 * j:2 * j + 2,
                                       m * 128:(m + 1) * 128],
                                  h1p3[:, 2 * j:2 * j + 2, :],
                                  start=(j == 0), stop=(j == 1),
                                  perf_mode=DRMODE)
                    sc.activation(h2p[:, m * NB:(m + 1) * NB], pp[:, :],
                                  AF.Gelu, scale=ISC8, bias=pb2_sb[:, m:m + 1])
                h2f = bigp.tile([128, 4 * NB], F8, tag="h2f")
                h1f3 = h1f[:, :].rearrange("p (k n) -> p k n", k=4)
                fw23 = fw2_sb[:, :].rearrange("p (k m) -> p k m", k=4)
                for m in range(4):
                    pp = psp.tile([128, NB], F32, tag="ps_main")
                    for j in range(2):
                        te.matmul(pp[:, :],
                                  fw23[:, 2 * j:2 * j + 2,
                                       m * 128:(m + 1) * 128],
                                  h1f3[:, 2 * j:2 * j + 2, :],
                                  start=(j == 0), stop=(j == 1),
                                  perf_mode=DRMODE)
                    sc.activation(h2f[:, m * NB:(m + 1) * NB], pp[:, :],
                                  AF.Gelu, scale=ISC8, bias=fb2_sb[:, m:m + 1])

                # layer 3 (padded to M=32; pres in col-group 0, fe in
                # col-group 1 of the same psum tile -> MMs run concurrently)
                # L3 merged: logit in M-col 0, pf/pe in M-cols 1:3 of one
                # zero-padded [32, NB] psum accumulation group
                p3 = ps3.tile([32, NB], F32, tag="ps3")
                pw33 = pw3_sb[:, :].rearrange("p (k m) -> p k m", k=2)
                fw33 = fw3_sb[:, :].rearrange("p (k m) -> p k m", k=4)
                h2p3 = h2p[:, :].rearrange("p (k n) -> p k n", k=2)
                h2f3 = h2f[:, :].rearrange("p (k n) -> p k n", k=4)
                te.matmul(p3[:, :], pw33[:, 0:2, :], h2p3[:, 0:2, :],
                          start=True, stop=False, perf_mode=DRMODE)
                for j in range(2):
                    te.matmul(p3[:, :], fw33[:, 2 * j:2 * j + 2, :],
                              h2f3[:, 2 * j:2 * j + 2, :],
                              start=False, stop=(j == 1), perf_mode=DRMODE)

                # bridge back to blocked batch-major; add b3 biases after
                lgT = smp.tile([32, NBLK * 32], F32, tag="lgT")
                v.transpose(lgT[:, :], p3[:, :])
                lg3 = r3(lgT, 32)
                fe3 = lg3
                r9v = r3(raw9[mt], 9)
                logit = r9v[:, :, s:s + 1]
                pf = r9v[:, :, 3 + s:4 + s]
                pe = r9v[:, :, 6 + s:7 + s]
                v.tensor_scalar(logit, lg3[:, :, 0:1], ISC8, b3bc[:, 0:1],
                                ALU.mult, ALU.add)
                v.tensor_scalar(pf, fe3[:, :, 1:2], ISC8, b3bc[:, 1:2],
                                ALU.mult, ALU.add)
                v.tensor_scalar(pe, fe3[:, :, 2:3], ISC8, b3bc[:, 2:3],
                                ALU.mult, ALU.add)

                # ---- plumbing: acts slots 0:3 (p,f,e), preds 3:6 --------
                pb = smp.tile([32, NBLK * 8], F32, tag="pb")
                pb3d = r3(pb, 8)
                acts = pb3d[:, :, 0:3]
                sig, pfc, pec = pb3d[:, :, 3:4], pb3d[:, :, 4:5], pb3d[:, :, 5:6]

                # sigmoid(l) = 0.5*tanh(0.5*l) + 0.5 (stays on the gelu table)
                sc.activation(sig, logit, AF.Tanh, scale=0.5)
                v.tensor_scalar(sig, sig, 0.5, 0.5, ALU.mult, ALU.add)
                v.tensor_scalar(pfc, pf, -10.0, 10.0, ALU.max, ALU.min)
                v.tensor_scalar(pec, pe, -100.0, 100.0, ALU.max, ALU.min)
                v.tensor_copy(acts, gt3[:, :, :])
                v.copy_predicated(acts, mk3[:, :, :], pb3d[:, :, 3:6])

                # state scatter: one 3-wide predicated copy per round, flags
                # via ones under a stride-3 view of the 9-wide round one-hot
                for r in range(3):
                    v.copy_predicated(st3[:, :, 3 * r:3 * r + 3],
                                      roi9[:, :, 3 * r:3 * r + 3], acts)
                v.copy_predicated(st3[:, :, S_FL:S_FL + 3],
                                  roi9[:, :, 0:9:3], r3(ones3, 3)[:, :, :])

                if s == 2:
                    rows = slice(mt * NB, (mt + 1) * NB)
                    out9 = smp.tile([32, NBLK * 9], F32, tag="out9")
                    v.tensor_copy(r3(out9, 9)[:, :, :], st3[:, :, 0:9])
                    nc.sync.dma_start(
                        dpfe_d[rows, :].rearrange("(j p) r -> p j r", p=32),
                        r3(out9, 9)[:, :, :])
                    nc.gpsimd.dma_start(
                        raw9_d[rows, :].rearrange("(j p) r -> p j r", p=32),
                        r9v[:, :, :])

        for p in (ps3, psp, smp, stp, bigp, pers, wpool):
            p.release()

    nc.compile()
    return nc


# ---------------------------------------------------------------------------
def prep_inputs(seq_embed, freq, pres, enrich,
                pw1, pb1, pw2, pb2, pw3, pb3,
                fw1, fb1, fw2, fb2, fw3, fb3,
                perm_idx, round_mask, BL):
    """Host-side (numpy) sharding + index preprocessing."""
    f32 = np.float32
    seq = np.asarray(seq_embed, f32)
    perms = ALL_PERMS[np.asarray(perm_idx)]                    # [B,3]
    gtf = np.take_along_axis(np.asarray(freq, f32), perms, 1)   # [B,3] (col=s)
    gtp = np.take_along_axis(np.asarray(pres, f32), perms, 1)
    gte = np.take_along_axis(np.asarray(enrich, f32), perms, 1)
    m = np.take_along_axis(np.asarray(round_mask), perms, 1).astype(f32)
    roh = (perms[:, :, None] == np.arange(3)[None, None, :]).astype(f32)  # [B,3s,3r]

    bf = lambda a: np.ascontiguousarray(np.asarray(a, f32).astype(NP_BF16))
    f8s = lambda a: np.ascontiguousarray(
        (np.asarray(a, f32) * SC8).astype(NP_F8))
    # extras rows in r-major slot order (p,f,e per round, flags, roh)
    pw1a = np.asarray(pw1, f32); fw1a = np.asarray(fw1, f32)
    pw1x = np.zeros((15, H), f32)
    for r in range(3):
        pw1x[3 * r] = pw1a[512 + 2 * r]          # pres_r
        pw1x[9 + r] = pw1a[513 + 2 * r]          # flag_r
        pw1x[12 + r] = pw1a[518 + r]             # roh_r
    fw1x = np.zeros((15, H), f32)
    for r in range(3):
        fw1x[3 * r + 0] = fw1a[513 + 4 * r]      # pres_r
        fw1x[3 * r + 1] = fw1a[512 + 4 * r]      # freq_r
        fw1x[3 * r + 2] = fw1a[514 + 4 * r]      # enrich_r
        fw1x[9 + r] = fw1a[515 + 4 * r]          # flag_r
        fw1x[12 + r] = fw1a[524 + r]             # roh_r
    w1xq = np.zeros((128, H), f32)
    w1xq[0:15] = pw1x; w1xq[32:47] = pw1x
    w1xq[64:79] = fw1x; w1xq[96:111] = fw1x
    w1xq = bf(w1xq)
    pw3p = np.zeros((256, 32), f32); pw3p[:, 0] = np.asarray(pw3, f32)[:, 0]
    fw3p = np.zeros((512, 32), f32); fw3p[:, 1:3] = np.asarray(fw3, f32)
    b3s = np.array([[np.asarray(pb3, f32)[0],
                     np.asarray(fb3, f32)[0], np.asarray(fb3, f32)[1]]], f32)

    shared = {
        "pw1": f8s(np.asarray(pw1, f32)[:512]), "w1xq": w1xq,
        "pb1": np.ascontiguousarray(np.asarray(pb1, f32)),
        "pw2": np.ascontiguousarray((np.asarray(pw2, f32) * SC8).astype(NP_F8)),
        "pb2": np.ascontiguousarray(np.asarray(pb2, f32)),
        "pw3p": f8s(pw3p),
        "fw1": f8s(np.asarray(fw1, f32)[:512]),
        "fb1": np.ascontiguousarray(np.asarray(fb1, f32)),
        "fw2": np.ascontiguousarray((np.asarray(fw2, f32) * SC8).astype(NP_F8)),
        "fb2": np.ascontiguousarray(np.asarray(fb2, f32)),
        "fw3p": f8s(fw3p),
        "eye": np.eye(128, dtype=NP_BF16),
        "b3s": b3s,
    }

    in_maps = []
    ncores = seq.shape[0] // BL
    BLKT = BL // 32
    for c in range(ncores):
        rs = slice(c * BL, (c + 1) * BL)
        # blocked layouts: index [s*32+p, Jg*w + q], b_local = 32*Jg + p
        gt3 = np.stack([gtp[rs], gtf[rs], gte[rs]], -1)          # [BL,3s,3]
        gt3 = gt3.reshape(BLKT, 32, 3, 3).transpose(2, 1, 0, 3)  # [3s,32,J,3]
        rohc = roh[rs].reshape(BLKT, 32, 3, 3).transpose(2, 1, 0, 3)
        mk3 = np.repeat(m[rs][:, :, None], 3, axis=2)
        mk3 = mk3.reshape(BLKT, 32, 3, 3).transpose(2, 1, 0, 3)
        r9 = np.repeat(roh[rs], 3, axis=2)                       # [BL,3s,9]
        r9 = r9.reshape(BLKT, 32, 3, 9).transpose(2, 1, 0, 3)
        in_maps.append(dict(
            seq=np.ascontiguousarray(seq[rs].astype(NP_F8).T),
            gts=np.ascontiguousarray(gt3.reshape(3 * 32, BLKT * 3)),
            roh=np.ascontiguousarray(rohc.reshape(3 * 32, BLKT * 3)),
            mski=np.ascontiguousarray(
                mk3.reshape(3 * 32, BLKT * 3).astype(np.uint8)),
            roi9=np.ascontiguousarray(
                r9.reshape(3 * 32, BLKT * 9).astype(np.uint8)),
            **shared))
    aux = dict(gtf=gtf, gtp=gtp, gte=gte, m=m)
    return in_maps, aux


def assemble(results, aux):
    """Gather per-core outputs; finish the (tiny) loss reductions host-side."""
    f32 = np.float32
    dpfe = np.concatenate([r["dpfe"] for r in results], 0).astype(f32)
    dpfe = dpfe.reshape(-1, 3, 3)
    dp, df, de = dpfe[:, :, 0], dpfe[:, :, 1], dpfe[:, :, 2]
    r9 = np.concatenate([r["raw9"] for r in results], 0).astype(f32)
    lg, pf, pe = r9[:, 0:3], r9[:, 3:6], r9[:, 6:9]
    m, gtf, gtp, gte = aux["m"], aux["gtf"], aux["gtp"], aux["gte"]
    lf = np.sum(np.square(pf - gtf) * m, dtype=np.float64)
    le = np.sum(np.square(pe - gte) * m, dtype=np.float64)
    bce = (np.maximum(lg, 0.0) - lg * gtp
           + np.log1p(np.exp(-np.abs(lg), dtype=np.float64)))
    lp = np.sum(bce * m, dtype=np.float64)
    nm = np.sum(m, dtype=np.float64) + 1e-8
    head = np.array([lf / nm, lp / nm, le / nm], f32)
    return np.concatenate([head, df.ravel(), dp.ravel(), de.ravel()])


_CACHE = {}


def _get_graph(BL):
    if BL not in _CACHE:
        _CACHE[BL] = build_graph(BL)
    return _CACHE[BL]


def _install_profile_hook():
    """Provide antenv.axon_hooks (missing in this image) so trace=True works."""
    import sys, types
    try:
        import antenv.axon_hooks  # noqa: F401
        return
    except ImportError:
        pass
    from trn_agent_boot.trn_boot import _ntff_profile_via_ctypes
    hook = _ntff_profile_via_ctypes('/opt/axon/libaxon_pjrt.so')
    mod = types.ModuleType('antenv.axon_hooks')
    mod._hook = hook
    mod.get_axon_ntff_profile_hook = lambda: mod._hook
    mod.set_axon_ntff_profile_hook = lambda h: setattr(mod, '_hook', h)
    sys.modules['antenv.axon_hooks'] = mod


def run(inputs, trace=False):
    if trace:
        _install_profile_hook()
    BL = inputs["seq_embed"].shape[0] // NCORES
    nc = _get_graph(BL)
    in_maps, aux = prep_inputs(**inputs, BL=BL)
    res = run_bass_kernel_spmd(nc, in_maps, core_ids=list(range(NCORES)),
                               trace=trace)
    out = assemble(res.results, aux)
    return out, res


def kernel(**inputs):
    inputs = {k: np.asarray(v) for k, v in inputs.items()}
    out, _ = run(inputs)
    return out



# revision 27
# speedup vs baseline: 1.1681x; 1.1681x over previous
"""Trainium2 Bass kernel for nn_AutoregressiveDecoder (8-core data parallel).

Strategy:
  - Pure data parallel: B=16384 rows sharded 2048/core across 8 NeuronCores.
  - MLP compute runs feature-major (features on partitions, batch on the free
    dim) so weights act as the matmul stationary operand.
  - The big matmuls (seq@w1, h1@w2, h2@w3) run as fp8-e4m3 DoubleRow
    (K=256/instruction). Weights are host-scaled by 64 so fp8 sees
    ~unit-variance values; the 1/64 descale folds into the consumer ACT
    `scale` (or the post-L3 affine). h1/h2 activations are written fp8 by
    the gelu itself; bases and state stay bf16/f32.
  - seq_embed @ w1[:512] is step-invariant -> computed once per 512-row
    macro-tile ("base", bf16), per-step the small state/onehot extras
    (K=9/15, quad-packed into all four PE row-groups) are matmul'd and the
    base re-added via a bf16 identity matmul in the same PSUM group.
  - L3 merges both heads into one zero-padded [32, NB] PSUM accumulation
    (logit col 0, freq/enrich cols 1:2) -> single StreamTranspose back, with
    descale+bias fused into the tensor_scalar that writes the raw-output
    tiles (also the host-side loss inputs).
  - Per-row scalar plumbing (sigmoid via tanh, clips, selects, state
    scatter) runs in a blocked batch-major layout [32 partitions,
    16 blocks x 32 slots], bridged with 32x32 StreamTransposes on DVE.
  - Index-only preprocessing (ALL_PERMS lookup, one-hot, take_along_axis
    gathers) happens host-side in numpy; loss sums are finished host-side
    from the raw logits/preds (psum of scalars x 8 cores).
"""

import numpy as np
import ml_dtypes

import concourse.bass as bass
import concourse.bacc as bacc
import concourse.tile as tile
from concourse import mybir
from concourse.bass_utils import run_bass_kernel_spmd

BF16 = mybir.dt.bfloat16
F8 = mybir.dt.float8e4
F32 = mybir.dt.float32
AF = mybir.ActivationFunctionType
ALU = mybir.AluOpType
DRMODE = mybir.MatmulPerfMode.DoubleRow
NP_BF16 = ml_dtypes.bfloat16
NP_F8 = ml_dtypes.float8_e4m3
SC8 = 64.0
ISC8 = 1.0 / SC8

B, D, H = 16384, 512, 512
NCORES = 8
NB = 512            # macro-tile rows (matmul free dim)
ALL_PERMS = np.array(
    [[0, 1, 2], [0, 2, 1], [1, 0, 2], [1, 2, 0], [2, 0, 1], [2, 1, 0]], np.int32
)

# blocked-layout slot map (32 slots per 32-row block), r-major:
# slot 3r+0 = pres_r, 3r+1 = freq_r, 3r+2 = enrich_r; 9:12 flags; 12:15 roh
S_FL, S_ROH = 9, 12


def r3(t, s):
    """view a [32, 16*s] tile as [32 p, 16 j, s slots]"""
    return t[:, :].rearrange("p (j s) -> p j s", s=s)


def _enable_ldw_opt():
    """walrus --enable-ldw-opt=false is hardcoded; flip it (dedups LDWEIGHTS)."""
    from concourse import bass_utils as bu
    if getattr(bu, "_ldw_patched", False):
        return
    orig = bu.run_command

    def patched(cmd, *a, **k):
        cmd = list(cmd)  # ldw-opt=true crashes walrus on this BIR; keep off
        return orig(cmd, *a, **k)

    bu.run_command = patched
    bu._ldw_patched = True


def build_graph(BL):
    """Build the per-core Bass graph. BL = rows per core (multiple of NB)."""
    _enable_ldw_opt()
    NM = BL // NB          # macro-tiles per core
    NBLK = NB // 32        # 32-row blocks per macro-tile (16)
    BLKT = BL // 32        # total blocks per core

    nc = bacc.Bacc("TRN2", target_bir_lowering=False, debug=False,
                   num_devices=NCORES)

    # ---- dram parameters -------------------------------------------------
    U8 = mybir.dt.uint8
    seq_d = nc.dram_tensor("seq", [D, BL], F8, kind="ExternalInput").ap()
    gts_d = nc.dram_tensor("gts", [96, BLKT * 3], F32, kind="ExternalInput").ap()
    roh_d = nc.dram_tensor("roh", [96, BLKT * 3], F32, kind="ExternalInput").ap()
    # uint8 copies of the masks (CopyPredicated wants integer predicates)
    mski_d = nc.dram_tensor("mski", [96, BLKT * 3], U8, kind="ExternalInput").ap()
    roi9_d = nc.dram_tensor("roi9", [96, BLKT * 9], U8, kind="ExternalInput").ap()

    pw1_d = nc.dram_tensor("pw1", [D, H], F8, kind="ExternalInput").ap()
    w1xq_d = nc.dram_tensor("w1xq", [128, H], BF16, kind="ExternalInput").ap()
    pb1_d = nc.dram_tensor("pb1", [H], F32, kind="ExternalInput").ap()
    pw2_d = nc.dram_tensor("pw2", [H, H // 2], F8, kind="ExternalInput").ap()
    pb2_d = nc.dram_tensor("pb2", [H // 2], F32, kind="ExternalInput").ap()
    pw3_d = nc.dram_tensor("pw3p", [H // 2, 32], F8, kind="ExternalInput").ap()

    fw1_d = nc.dram_tensor("fw1", [D, H], F8, kind="ExternalInput").ap()
    fb1_d = nc.dram_tensor("fb1", [H], F32, kind="ExternalInput").ap()
    fw2_d = nc.dram_tensor("fw2", [H, H], F8, kind="ExternalInput").ap()
    fb2_d = nc.dram_tensor("fb2", [H], F32, kind="ExternalInput").ap()
    fw3_d = nc.dram_tensor("fw3p", [H, 32], F8, kind="ExternalInput").ap()
    eye_d = nc.dram_tensor("eye", [128, 128], BF16, kind="ExternalInput").ap()
    b3s_d = nc.dram_tensor("b3s", [1, 3], F32, kind="ExternalInput").ap()

    dpfe_d = nc.dram_tensor("dpfe", [BL, 9], F32, kind="ExternalOutput").ap()
    # raw per-step MLP outputs (loss terms are finished host-side)
    raw9_d = nc.dram_tensor("raw9", [BL, 9], F32, kind="ExternalOutput").ap()

    v = nc.vector
    sc = nc.scalar
    te = nc.tensor

    with tile.TileContext(nc) as tc:
        wpool = tc.alloc_tile_pool(name="w", bufs=1)
        pers = tc.alloc_tile_pool(name="pers", bufs=1)
        bigp = tc.alloc_tile_pool(name="big", bufs=3)
        stp = tc.alloc_tile_pool(name="stp", bufs=2)
        smp = tc.alloc_tile_pool(name="smp", bufs=4)
        psp = tc.alloc_tile_pool(name="psum", bufs=6, space="PSUM")
        ps3 = tc.alloc_tile_pool(name="psum3", bufs=2, space="PSUM")

        # ---- load weights (persistent) ----------------------------------
        pw1_sb = wpool.tile([128, 4 * H], F8)
        fw1_sb = wpool.tile([128, 4 * H], F8)
        pw2_sb = wpool.tile([128, 4 * (H // 2)], F8)
        fw2_sb = wpool.tile([128, 4 * H], F8)
        for k in range(4):
            nc.gpsimd.dma_start(pw1_sb[:, k * H:(k + 1) * H],
                                pw1_d[k * 128:(k + 1) * 128, :])
            nc.gpsimd.dma_start(fw1_sb[:, k * H:(k + 1) * H],
                                fw1_d[k * 128:(k + 1) * 128, :])
        # biases as [128, nchunk] f32 (partition p, chunk m) for ACT bias APs
        pb1_sb = wpool.tile([128, 4], F32)
        fb1_sb = wpool.tile([128, 4], F32)
        fb2_sb = wpool.tile([128, 4], F32)
        pb2_sb = wpool.tile([128, 2], F32)
        nc.scalar.dma_start(pb1_sb[:, :],
                            pb1_d.rearrange("(m p) -> p m", p=128))
        nc.scalar.dma_start(fb1_sb[:, :],
                            fb1_d.rearrange("(m p) -> p m", p=128))
        for k in range(4):
            nc.gpsimd.dma_start(pw2_sb[:, k * 256:(k + 1) * 256],
                                pw2_d[k * 128:(k + 1) * 128, :])
            nc.gpsimd.dma_start(fw2_sb[:, k * H:(k + 1) * H],
                                fw2_d[k * 128:(k + 1) * 128, :])
        nc.scalar.dma_start(fb2_sb[:, :],
                            fb2_d.rearrange("(m p) -> p m", p=128))
        nc.scalar.dma_start(pb2_sb[:, :],
                            pb2_d.rearrange("(m p) -> p m", p=128))
        w1xq_sb = wpool.tile([128, H], BF16)
        nc.scalar.dma_start(w1xq_sb[:, :], w1xq_d[:, :])
        pw3_sb = wpool.tile([128, 64], F8)   # 2 k-chunks x 32 cols
        fw3_sb = wpool.tile([128, 128], F8)  # 4 k-chunks x 32 cols
        for k in range(2):
            nc.scalar.dma_start(pw3_sb[:, k * 32:(k + 1) * 32],
                                pw3_d[k * 128:(k + 1) * 128, :])
        for k in range(4):
            nc.scalar.dma_start(fw3_sb[:, k * 32:(k + 1) * 32],
                                fw3_d[k * 128:(k + 1) * 128, :])
        eye_sb = wpool.tile([128, 128], BF16)
        nc.scalar.dma_start(eye_sb[:, :], eye_d[:, :])
        # l3 output biases broadcast to 32 partitions: cols (pb3, fb3_0, fb3_1)
        b3s_sb = wpool.tile([1, 3], F32)
        nc.scalar.dma_start(b3s_sb[:, :], b3s_d[:, :])
        b3bc = wpool.tile([32, 3], F32)
        nc.gpsimd.partition_broadcast(b3bc[:, :], b3s_sb[:, :])
        ones3 = wpool.tile([32, NBLK * 3], F32)
        v.memset(ones3[:, :], 1.0)

        # PE warm-up: ~100 tiny matmuls on a zeroed tile during the DMA
        # preamble so phase 1 starts at the full 2.4 GHz HAM clock
        warm = wpool.tile([128, 128], BF16)
        v.memset(warm[:, :], 0.0)
        pwarm = psp.tile([128, NB], F32, tag="ps_main")
        for _ in range(40):
            te.matmul(pwarm[:, 0:128], warm[:, :], warm[:, :],
                      start=True, stop=True)

        # ---- phase 1: all macro-tile bases (PE-dense, keeps HAM warm) ---
        base_p, base_f, sts, raw9 = {}, {}, {}, {}
        pw13 = pw1_sb[:, :].rearrange("p (k m) -> p k m", k=4)
        fw13 = fw1_sb[:, :].rearrange("p (k m) -> p k m", k=4)
        for mt in range(NM):
            rows = slice(mt * NB, (mt + 1) * NB)
            seqT = stp.tile([128, 4 * NB], F8, tag="seqT", name=f"seqT{mt}")
            sq3 = seqT[:, :].rearrange("p (k n) -> p k n", k=4)
            nc.sync.dma_start(
                sq3[:, :, :],
                seq_d[:, rows].rearrange("(k p) n -> p k n", k=4))
            base_p[mt] = pers.tile([128, 4 * NB], BF16, tag=f"base_p{mt}", name=f"base_p{mt}")
            base_f[mt] = pers.tile([128, 4 * NB], BF16, tag=f"base_f{mt}", name=f"base_f{mt}")
            for m in range(4):
                pp = psp.tile([128, NB], F32, tag="ps_main")
                for j in range(2):
                    te.matmul(pp[:, :],
                              pw13[:, 2 * j:2 * j + 2, m * 128:(m + 1) * 128],
                              sq3[:, 2 * j:2 * j + 2, :],
                              start=(j == 0), stop=(j == 1),
                              perf_mode=DRMODE)
                v.tensor_scalar(base_p[mt][:, m * NB:(m + 1) * NB], pp[:, :],
                                ISC8, pb1_sb[:, m:m + 1], ALU.mult, ALU.add)
                pf_ = psp.tile([128, NB], F32, tag="ps_main")
                for j in range(2):
                    te.matmul(pf_[:, :],
                              fw13[:, 2 * j:2 * j + 2, m * 128:(m + 1) * 128],
                              sq3[:, 2 * j:2 * j + 2, :],
                              start=(j == 0), stop=(j == 1),
                              perf_mode=DRMODE)
                v.tensor_scalar(base_f[mt][:, m * NB:(m + 1) * NB], pf_[:, :],
                                ISC8, fb1_sb[:, m:m + 1], ALU.mult, ALU.add)
            sts[mt] = pers.tile([32, NBLK * 32], F32, tag=f"st{mt}", name=f"st{mt}")
            v.memset(sts[mt][:, :], 0.0)
            raw9[mt] = pers.tile([32, NBLK * 9], F32, tag=f"r9{mt}",
                                 name=f"r9{mt}")

        # ---- phase 2: autoregressive steps, macro-interleaved -----------
        # While macro mt's plumbing runs on DVE/ACT, the PE executes the
        # other macros' matmuls (engines are in-order; interleaved emission
        # is what lets the scheduler fill the gaps).
        for s in range(3):
            for mt in range(NM):
                st3 = r3(sts[mt], 32)
                gt_sb = smp.tile([32, NBLK * 3], F32, tag="gt")
                nc.sync.dma_start(
                    gt_sb[:, :],
                    gts_d[s * 32:(s + 1) * 32,
                          mt * NBLK * 3:(mt + 1) * NBLK * 3])
                gt3 = r3(gt_sb, 3)
                mk_sb = smp.tile([32, NBLK * 3], U8, tag="mi")
                nc.sync.dma_start(
                    mk_sb[:, :],
                    mski_d[s * 32:(s + 1) * 32,
                           mt * NBLK * 3:(mt + 1) * NBLK * 3])
                mk3 = r3(mk_sb, 3)
                roi_sb = smp.tile([32, NBLK * 9], U8, tag="roi")
                nc.sync.dma_start(
                    roi_sb[:, :],
                    roi9_d[s * 32:(s + 1) * 32,
                           mt * NBLK * 9:(mt + 1) * NBLK * 9])
                roi9 = r3(roi_sb, 9)
                roh_sb = smp.tile([32, NBLK * 3], F32, tag="rohs")
                nc.sync.dma_start(
                    roh_sb[:, :],
                    roh_d[s * 32:(s + 1) * 32,
                          mt * NBLK * 3:(mt + 1) * NBLK * 3])
                roh3 = r3(roh_sb, 3)

                # bridge to feature-major: cast + stream-transpose
                # (st slots 6..8 stay zero; the one-hot joins here)
                st_bf = smp.tile([32, NBLK * 32], BF16, tag="stbf")
                v.tensor_copy(st_bf[:, :], sts[mt][:, :])
                v.tensor_copy(r3(st_bf, 32)[:, :, S_ROH:S_ROH + 3],
                              roh3[:, :, :])  # slots 12:15
                exT = smp.tile([128, NBLK * 32], BF16, tag="exT")
                v.transpose(exT[0:32, :], st_bf[:, :])
                v.tensor_copy(exT[32:47, :], exT[0:15, :])
                v.tensor_copy(exT[64:79, :], exT[0:15, :])
                v.tensor_copy(exT[96:111, :], exT[0:15, :])

                # layer 1: base + extra @ w1x -> gelu -> h1 (bf16)
                h1p = bigp.tile([128, 4 * NB], F8, tag="h1p")
                h1f = bigp.tile([128, 4 * NB], F8, tag="h1f")
                for half in range(2):
                    quad = []
                    for i in range(2):
                        m = 2 * half + i
                        pp = psp.tile([128, NB], F32, tag="ps_main")
                        pf_ = psp.tile([128, NB], F32, tag="ps_main")
                        te.matmul(pp[:, :],
                                  w1xq_sb[32 * i:32 * i + 15,
                                          m * 128:(m + 1) * 128],
                                  exT[32 * i:32 * i + 15, :],
                                  start=True, stop=False,
                                  tile_position=(32 * i, 0))
                        te.matmul(pf_[:, :],
                                  w1xq_sb[64 + 32 * i:64 + 32 * i + 15,
                                          m * 128:(m + 1) * 128],
                                  exT[64 + 32 * i:64 + 32 * i + 15, :],
                                  start=True, stop=False,
                                  tile_position=(64 + 32 * i, 0))
                        quad.append((m, pp, pf_))
                    for m, pp, pf_ in quad:
                        te.matmul(pp[:, :], eye_sb[:, :],
                                  base_p[mt][:, m * NB:(m + 1) * NB],
                                  start=False, stop=True)
                        te.matmul(pf_[:, :], eye_sb[:, :],
                                  base_f[mt][:, m * NB:(m + 1) * NB],
                                  start=False, stop=True)
                        sc.activation(h1p[:, m * NB:(m + 1) * NB], pp[:, :],
                                      AF.Gelu)
                        sc.activation(h1f[:, m * NB:(m + 1) * NB], pf_[:, :],
                                      AF.Gelu)

                # layer 2
                h2p = bigp.tile([128, 2 * NB], F8, tag="h2p")
                h1p3 = h1p[:, :].rearrange("p (k n) -> p k n", k=4)
                pw23 = pw2_sb[:, :].rearrange("p (k m) -> p k m", k=4)
                for m in range(2):
                    pp = psp.tile([128, NB], F32, tag="ps_main")
                    for j in range(2):
                        te.matmul(pp[:, :],
                                  pw23[:, 2 * j:2 * j + 2,
                                       m * 128:(m + 1) * 128],
                                  h1p3[:, 2 * j:2 * j + 2, :],
                                  start=(j == 0), stop=(j == 1),
                                  perf_mode=DRMODE)
                    sc.activation(h2p[:, m * NB:(m + 1) * NB], pp[:, :],
                                  AF.Gelu, scale=ISC8, bias=pb2_sb[:, m:m + 1])
                h2f = bigp.tile([128, 4 * NB], F8, tag="h2f")
                h1f3 = h1f[:, :].rearrange("p (k n) -> p k n", k=4)
                fw23 = fw2_sb[:, :].rearrange("p (k m) -> p k m", k=4)
                for m in range(4):
                    pp = psp.tile([128, NB], F32, tag="ps_main")
                    for j in range(2):
                        te.matmul(pp[:, :],
                                  fw23[:, 2 * j:2 * j + 2,
                                       m * 128:(m + 1) * 128],
                                  h1f3[:, 2 * j:2 * j + 2, :],
                                  start=(j == 0), stop=(j == 1),
                                  perf_mode=DRMODE)
                    sc.activation(h2f[:, m * NB:(m + 1) * NB], pp[:, :],
                                  AF.Gelu, scale=ISC8, bias=fb2_sb[:, m:m + 1])

                # layer 3 (padded to M=32; pres in col-group 0, fe in
                # col-group 1 of the same psum tile -> MMs run concurrently)
                # L3 merged: logit in M-col 0, pf/pe in M-cols 1:3 of one
                # zero-padded [32, NB] psum accumulation group
                p3 = ps3.tile([32, NB], F32, tag="ps3")
                pw33 = pw3_sb[:, :].rearrange("p (k m) -> p k m", k=2)
                fw33 = fw3_sb[:, :].rearrange("p (k m) -> p k m", k=4)
                h2p3 = h2p[:, :].rearrange("p (k n) -> p k n", k=2)
                h2f3 = h2f[:, :].rearrange("p (k n) -> p k n", k=4)
                te.matmul(p3[:, :], pw33[:, 0:2, :], h2p3[:, 0:2, :],
                          start=True, stop=False, perf_mode=DRMODE)
                for j in range(2):
                    te.matmul(p3[:, :], fw33[:, 2 * j:2 * j + 2, :],
                              h2f3[:, 2 * j:2 * j + 2, :],
                              start=False, stop=(j == 1), perf_mode=DRMODE)

                # bridge back to blocked batch-major; add b3 biases after
                lgT = smp.tile([32, NBLK * 32], F32, tag="lgT")
                v.transpose(lgT[:, :], p3[:, :])
                lg3 = r3(lgT, 32)
                fe3 = lg3
                r9v = r3(raw9[mt], 9)
                logit = r9v[:, :, s:s + 1]
                pf = r9v[:, :, 3 + s:4 + s]
                pe = r9v[:, :, 6 + s:7 + s]
                v.tensor_scalar(logit, lg3[:, :, 0:1], ISC8, b3bc[:, 0:1],
                                ALU.mult, ALU.add)
                v.tensor_scalar(pf, fe3[:, :, 1:2], ISC8, b3bc[:, 1:2],
                                ALU.mult, ALU.add)
                v.tensor_scalar(pe, fe3[:, :, 2:3], ISC8, b3bc[:, 2:3],
                                ALU.mult, ALU.add)

                # ---- plumbing: acts slots 0:3 (p,f,e), preds 3:6 --------
                pb = smp.tile([32, NBLK * 8], F32, tag="pb")
                pb3d = r3(pb, 8)
                acts = pb3d[:, :, 0:3]
                sig, pfc, pec = pb3d[:, :, 3:4], pb3d[:, :, 4:5], pb3d[:, :, 5:6]

                # sigmoid(l) = 0.5*tanh(0.5*l) + 0.5 (stays on the gelu table)
                sc.activation(sig, logit, AF.Tanh, scale=0.5)
                v.tensor_scalar(sig, sig, 0.5, 0.5, ALU.mult, ALU.add)
                v.tensor_scalar(pfc, pf, -10.0, 10.0, ALU.max, ALU.min)
                v.tensor_scalar(pec, pe, -100.0, 100.0, ALU.max, ALU.min)
                v.tensor_copy(acts, gt3[:, :, :])
                v.copy_predicated(acts, mk3[:, :, :], pb3d[:, :, 3:6])

                # state scatter: one 3-wide predicated copy per round, flags
                # via ones under a stride-3 view of the 9-wide round one-hot
                for r in range(3):
                    v.copy_predicated(st3[:, :, 3 * r:3 * r + 3],
                                      roi9[:, :, 3 * r:3 * r + 3], acts)
                v.copy_predicated(st3[:, :, S_FL:S_FL + 3],
                                  roi9[:, :, 0:9:3], r3(ones3, 3)[:, :, :])

                if s == 2:
                    rows = slice(mt * NB, (mt + 1) * NB)
                    out9 = smp.tile([32, NBLK * 9], F32, tag="out9")
                    v.tensor_copy(r3(out9, 9)[:, :, :], st3[:, :, 0:9])
                    nc.sync.dma_start(
                        dpfe_d[rows, :].rearrange("(j p) r -> p j r", p=32),
                        r3(out9, 9)[:, :, :])
                    nc.gpsimd.dma_start(
                        raw9_d[rows, :].rearrange("(j p) r -> p j r", p=32),
                        r9v[:, :, :])

        for p in (ps3, psp, smp, stp, bigp, pers, wpool):
            p.release()

    nc.compile()
    return nc


# ---------------------------------------------------------------------------
def prep_inputs(seq_embed, freq, pres, enrich,
                pw1, pb1, pw2, pb2, pw3, pb3,
                fw1, fb1, fw2, fb2, fw3, fb3,
                perm_idx, round_mask, BL):
    """Host-side (numpy) sharding + index preprocessing."""
    f32 = np.float32
    seq = np.asarray(seq_embed, f32)
    perms = ALL_PERMS[np.asarray(perm_idx)]                    # [B,3]
    gtf = np.take_along_axis(np.asarray(freq, f32), perms, 1)   # [B,3] (col=s)
    gtp = np.take_along_axis(np.asarray(pres, f32), perms, 1)
    gte = np.take_along_axis(np.asarray(enrich, f32), perms, 1)
    m = np.take_along_axis(np.asarray(round_mask), perms, 1).astype(f32)
    roh = (perms[:, :, None] == np.arange(3)[None, None, :]).astype(f32)  # [B,3s,3r]

    bf = lambda a: np.ascontiguousarray(np.asarray(a, f32).astype(NP_BF16))
    f8s = lambda a: np.ascontiguousarray(
        (np.asarray(a, f32) * SC8).astype(NP_F8))
    # extras rows in r-major slot order (p,f,e per round, flags, roh)
    pw1a = np.asarray(pw1, f32); fw1a = np.asarray(fw1, f32)
    pw1x = np.zeros((15, H), f32)
    for r in range(3):
        pw1x[3 * r] = pw1a[512 + 2 * r]          # pres_r
        pw1x[9 + r] = pw1a[513 + 2 * r]          # flag_r
        pw1x[12 + r] = pw1a[518 + r]             # roh_r
    fw1x = np.zeros((15, H), f32)
    for r in range(3):
        fw1x[3 * r + 0] = fw1a[513 + 4 * r]      # pres_r
        fw1x[3 * r + 1] = fw1a[512 + 4 * r]      # freq_r
        fw1x[3 * r + 2] = fw1a[514 + 4 * r]      # enrich_r
        fw1x[9 + r] = fw1a[515 + 4 * r]          # flag_r
        fw1x[12 + r] = fw1a[524 + r]             # roh_r
    w1xq = np.zeros((128, H), f32)
    w1xq[0:15] = pw1x; w1xq[32:47] = pw1x
    w1xq[64:79] = fw1x; w1xq[96:111] = fw1x
    w1xq = bf(w1xq)
    pw3p = np.zeros((256, 32), f32); pw3p[:, 0] = np.asarray(pw3, f32)[:, 0]
    fw3p = np.zeros((512, 32), f32); fw3p[:, 1:3] = np.asarray(fw3, f32)
    b3s = np.array([[np.asarray(pb3, f32)[0],
                     np.asarray(fb3, f32)[0], np.asarray(fb3, f32)[1]]], f32)

    shared = {
        "pw1": f8s(np.asarray(pw1, f32)[:512]), "w1xq": w1xq,
        "pb1": np.ascontiguousarray(np.asarray(pb1, f32)),
        "pw2": np.ascontiguousarray((np.asarray(pw2, f32) * SC8).astype(NP_F8)),
        "pb2": np.ascontiguousarray(np.asarray(pb2, f32)),
        "pw3p": f8s(pw3p),
        "fw1": f8s(np.asarray(fw1, f32)[:512]),
        "fb1": np.ascontiguousarray(np.asarray(fb1, f32)),
        "fw2": np.ascontiguousarray((np.asarray(fw2, f32) * SC8).astype(NP_F8)),
        "fb2": np.ascontiguousarray(np.asarray(fb2, f32)),
        "fw3p": f8s(fw3p),
        "eye": np.eye(128, dtype=NP_BF16),
        "b3s": b3s,
    }

    in_maps = []
    ncores = seq.shape[0] // BL
    BLKT = BL // 32
    for c in range(ncores):
        rs = slice(c * BL, (c + 1) * BL)
        # blocked layouts: index [s*32+p, Jg*w + q], b_local = 32*Jg + p
        gt3 = np.stack([gtp[rs], gtf[rs], gte[rs]], -1)          # [BL,3s,3]
        gt3 = gt3.reshape(BLKT, 32, 3, 3).transpose(2, 1, 0, 3)  # [3s,32,J,3]
        rohc = roh[rs].reshape(BLKT, 32, 3, 3).transpose(2, 1, 0, 3)
        mk3 = np.repeat(m[rs][:, :, None], 3, axis=2)
        mk3 = mk3.reshape(BLKT, 32, 3, 3).transpose(2, 1, 0, 3)
        r9 = np.repeat(roh[rs], 3, axis=2)                       # [BL,3s,9]
        r9 = r9.reshape(BLKT, 32, 3, 9).transpose(2, 1, 0, 3)
        in_maps.append(dict(
            seq=np.ascontiguousarray(seq[rs].astype(NP_F8).T),
            gts=np.ascontiguousarray(gt3.reshape(3 * 32, BLKT * 3)),
            roh=np.ascontiguousarray(rohc.reshape(3 * 32, BLKT * 3)),
            mski=np.ascontiguousarray(
                mk3.reshape(3 * 32, BLKT * 3).astype(np.uint8)),
            roi9=np.ascontiguousarray(
                r9.reshape(3 * 32, BLKT * 9).astype(np.uint8)),
            **shared))
    aux = dict(gtf=gtf, gtp=gtp, gte=gte, m=m)
    return in_maps, aux


def assemble(results, aux):
    """Gather per-core outputs; finish the (tiny) loss reductions host-side."""
    f32 = np.float32
    dpfe = np.concatenate([r["dpfe"] for r in results], 0).astype(f32)
    dpfe = dpfe.reshape(-1, 3, 3)
    dp, df, de = dpfe[:, :, 0], dpfe[:, :, 1], dpfe[:, :, 2]
    r9 = np.concatenate([r["raw9"] for r in results], 0).astype(f32)
    lg, pf, pe = r9[:, 0:3], r9[:, 3:6], r9[:, 6:9]
    m, gtf, gtp, gte = aux["m"], aux["gtf"], aux["gtp"], aux["gte"]
    lf = np.sum(np.square(pf - gtf) * m, dtype=np.float64)
    le = np.sum(np.square(pe - gte) * m, dtype=np.float64)
    bce = (np.maximum(lg, 0.0) - lg * gtp
           + np.log1p(np.exp(-np.abs(lg), dtype=np.float64)))
    lp = np.sum(bce * m, dtype=np.float64)
    nm = np.sum(m, dtype=np.float64) + 1e-8
    head = np.array([lf / nm, lp / nm, le / nm], f32)
    return np.concatenate([head, df.ravel(), dp.ravel(), de.ravel()])


_CACHE = {}


def _get_graph(BL):
    if BL not in _CACHE:
        _CACHE[BL] = build_graph(BL)
    return _CACHE[BL]


def _install_profile_hook():
    """Provide antenv.axon_hooks (missing in this image) so trace=True works."""
    import sys, types
    try:
        import antenv.axon_hooks  # noqa: F401
        return
    except ImportError:
        pass
    from trn_agent_boot.trn_boot import _ntff_profile_via_ctypes
    hook = _ntff_profile_via_ctypes('/opt/axon/libaxon_pjrt.so')
    mod = types.ModuleType('antenv.axon_hooks')
    mod._hook = hook
    mod.get_axon_ntff_profile_hook = lambda: mod._hook
    mod.set_axon_ntff_profile_hook = lambda h: setattr(mod, '_hook', h)
    sys.modules['antenv.axon_hooks'] = mod


def run(inputs, trace=False):
    if trace:
        _install_profile_hook()
    BL = inputs["seq_embed"].shape[0] // NCORES
    nc = _get_graph(BL)
    in_maps, aux = prep_inputs(**inputs, BL=BL)
    res = run_bass_kernel_spmd(nc, in_maps, core_ids=list(range(NCORES)),
                               trace=trace)
    out = assemble(res.results, aux)
    return out, res


def kernel(**inputs):
    inputs = {k: np.asarray(v) for k, v in inputs.items()}
    out, _ = run(inputs)
    return out



# revision 28
# speedup vs baseline: 1.1735x; 1.0046x over previous
"""Trainium2 Bass kernel for nn_AutoregressiveDecoder (8-core data parallel).

Strategy:
  - Pure data parallel: B=16384 rows sharded 2048/core across 8 NeuronCores.
  - MLP compute runs feature-major (features on partitions, batch on the free
    dim) so weights act as the matmul stationary operand.
  - The big matmuls (seq@w1, h1@w2, h2@w3) run as fp8-e4m3 DoubleRow
    (K=256/instruction). Weights are host-scaled by 64 so fp8 sees
    ~unit-variance values; the 1/64 descale folds into the consumer ACT
    `scale` (or the post-L3 affine). h1/h2 activations are written fp8 by
    the gelu itself; bases and state stay bf16/f32.
  - seq_embed @ w1[:512] is step-invariant -> computed once per 512-row
    macro-tile ("base", bf16), per-step the small state/onehot extras
    (K=9/15, quad-packed into all four PE row-groups) are matmul'd and the
    base re-added via a bf16 identity matmul in the same PSUM group.
  - L3 merges both heads into one zero-padded [32, NB] PSUM accumulation
    (logit col 0, freq/enrich cols 1:2) -> single StreamTranspose back, with
    descale+bias fused into the tensor_scalar that writes the raw-output
    tiles (also the host-side loss inputs).
  - Per-row scalar plumbing (sigmoid via tanh, clips, selects, state
    scatter) runs in a blocked batch-major layout [32 partitions,
    16 blocks x 32 slots], bridged with 32x32 StreamTransposes on DVE.
  - Index-only preprocessing (ALL_PERMS lookup, one-hot, take_along_axis
    gathers) happens host-side in numpy; loss sums are finished host-side
    from the raw logits/preds (psum of scalars x 8 cores).
"""

import numpy as np
import ml_dtypes

import concourse.bass as bass
import concourse.bacc as bacc
import concourse.tile as tile
from concourse import mybir
from concourse.bass_utils import run_bass_kernel_spmd

BF16 = mybir.dt.bfloat16
F8 = mybir.dt.float8e4
F32 = mybir.dt.float32
AF = mybir.ActivationFunctionType
ALU = mybir.AluOpType
DRMODE = mybir.MatmulPerfMode.DoubleRow
NP_BF16 = ml_dtypes.bfloat16
NP_F8 = ml_dtypes.float8_e4m3
SC8 = 64.0
ISC8 = 1.0 / SC8

B, D, H = 16384, 512, 512
NCORES = 8
NB = 512            # macro-tile rows (matmul free dim)
ALL_PERMS = np.array(
    [[0, 1, 2], [0, 2, 1], [1, 0, 2], [1, 2, 0], [2, 0, 1], [2, 1, 0]], np.int32
)

# blocked-layout slot map (32 slots per 32-row block), r-major:
# slot 3r+0 = pres_r, 3r+1 = freq_r, 3r+2 = enrich_r; 9:12 flags; 12:15 roh
S_FL, S_ROH = 9, 12


def r3(t, s):
    """view a [32, 16*s] tile as [32 p, 16 j, s slots]"""
    return t[:, :].rearrange("p (j s) -> p j s", s=s)


def _enable_ldw_opt():
    """walrus --enable-ldw-opt=false is hardcoded; flip it (dedups LDWEIGHTS)."""
    from concourse import bass_utils as bu
    if getattr(bu, "_ldw_patched", False):
        return
    orig = bu.run_command

    def patched(cmd, *a, **k):
        cmd = list(cmd)  # ldw-opt=true crashes walrus on this BIR; keep off
        return orig(cmd, *a, **k)

    bu.run_command = patched
    bu._ldw_patched = True


def build_graph(BL):
    """Build the per-core Bass graph. BL = rows per core (multiple of NB)."""
    _enable_ldw_opt()
    NM = BL // NB          # macro-tiles per core
    NBLK = NB // 32        # 32-row blocks per macro-tile (16)
    BLKT = BL // 32        # total blocks per core

    nc = bacc.Bacc("TRN2", target_bir_lowering=False, debug=False,
                   num_devices=NCORES)

    # ---- dram parameters -------------------------------------------------
    U8 = mybir.dt.uint8
    seq_d = nc.dram_tensor("seq", [D, BL], F8, kind="ExternalInput").ap()
    gts_d = nc.dram_tensor("gts", [96, BLKT * 3], F32, kind="ExternalInput").ap()
    roh_d = nc.dram_tensor("roh", [96, BLKT * 3], F32, kind="ExternalInput").ap()
    # uint8 copies of the masks (CopyPredicated wants integer predicates)
    mski_d = nc.dram_tensor("mski", [96, BLKT * 3], U8, kind="ExternalInput").ap()
    roi9_d = nc.dram_tensor("roi9", [96, BLKT * 9], U8, kind="ExternalInput").ap()

    pw1_d = nc.dram_tensor("pw1", [D, H], F8, kind="ExternalInput").ap()
    w1xq_d = nc.dram_tensor("w1xq", [128, H], BF16, kind="ExternalInput").ap()
    pb1_d = nc.dram_tensor("pb1", [H], F32, kind="ExternalInput").ap()
    pw2_d = nc.dram_tensor("pw2", [H, H // 2], F8, kind="ExternalInput").ap()
    pb2_d = nc.dram_tensor("pb2", [H // 2], F32, kind="ExternalInput").ap()
    pw3_d = nc.dram_tensor("pw3p", [H // 2, 32], F8, kind="ExternalInput").ap()

    fw1_d = nc.dram_tensor("fw1", [D, H], F8, kind="ExternalInput").ap()
    fb1_d = nc.dram_tensor("fb1", [H], F32, kind="ExternalInput").ap()
    fw2_d = nc.dram_tensor("fw2", [H, H], F8, kind="ExternalInput").ap()
    fb2_d = nc.dram_tensor("fb2", [H], F32, kind="ExternalInput").ap()
    fw3_d = nc.dram_tensor("fw3p", [H, 32], F8, kind="ExternalInput").ap()
    eye_d = nc.dram_tensor("eye", [128, 128], BF16, kind="ExternalInput").ap()
    b3s_d = nc.dram_tensor("b3s", [1, 3], F32, kind="ExternalInput").ap()

    dpfe_d = nc.dram_tensor("dpfe", [BL, 9], F32, kind="ExternalOutput").ap()
    # raw per-step MLP outputs (loss terms are finished host-side)
    raw9_d = nc.dram_tensor("raw9", [BL, 9], F32, kind="ExternalOutput").ap()

    v = nc.vector
    sc = nc.scalar
    te = nc.tensor

    with tile.TileContext(nc) as tc:
        wpool = tc.alloc_tile_pool(name="w", bufs=1)
        pers = tc.alloc_tile_pool(name="pers", bufs=1)
        bigp = tc.alloc_tile_pool(name="big", bufs=3)
        stp = tc.alloc_tile_pool(name="stp", bufs=2)
        smp = tc.alloc_tile_pool(name="smp", bufs=4)
        psp = tc.alloc_tile_pool(name="psum", bufs=6, space="PSUM")
        ps3 = tc.alloc_tile_pool(name="psum3", bufs=2, space="PSUM")

        # ---- load weights (persistent) ----------------------------------
        pw1_sb = wpool.tile([128, 4 * H], F8)
        fw1_sb = wpool.tile([128, 4 * H], F8)
        pw2_sb = wpool.tile([128, 4 * (H // 2)], F8)
        fw2_sb = wpool.tile([128, 4 * H], F8)
        for k in range(4):
            nc.gpsimd.dma_start(pw1_sb[:, k * H:(k + 1) * H],
                                pw1_d[k * 128:(k + 1) * 128, :])
            nc.gpsimd.dma_start(fw1_sb[:, k * H:(k + 1) * H],
                                fw1_d[k * 128:(k + 1) * 128, :])
        # biases as [128, nchunk] f32 (partition p, chunk m) for ACT bias APs
        pb1_sb = wpool.tile([128, 4], F32)
        fb1_sb = wpool.tile([128, 4], F32)
        fb2_sb = wpool.tile([128, 4], F32)
        pb2_sb = wpool.tile([128, 2], F32)
        nc.scalar.dma_start(pb1_sb[:, :],
                            pb1_d.rearrange("(m p) -> p m", p=128))
        nc.scalar.dma_start(fb1_sb[:, :],
                            fb1_d.rearrange("(m p) -> p m", p=128))
        for k in range(4):
            nc.gpsimd.dma_start(pw2_sb[:, k * 256:(k + 1) * 256],
                                pw2_d[k * 128:(k + 1) * 128, :])
            nc.gpsimd.dma_start(fw2_sb[:, k * H:(k + 1) * H],
                                fw2_d[k * 128:(k + 1) * 128, :])
        nc.scalar.dma_start(fb2_sb[:, :],
                            fb2_d.rearrange("(m p) -> p m", p=128))
        nc.scalar.dma_start(pb2_sb[:, :],
                            pb2_d.rearrange("(m p) -> p m", p=128))
        w1xq_sb = wpool.tile([128, H], BF16)
        nc.scalar.dma_start(w1xq_sb[:, :], w1xq_d[:, :])
        pw3_sb = wpool.tile([128, 64], F8)   # 2 k-chunks x 32 cols
        fw3_sb = wpool.tile([128, 128], F8)  # 4 k-chunks x 32 cols
        for k in range(2):
            nc.scalar.dma_start(pw3_sb[:, k * 32:(k + 1) * 32],
                                pw3_d[k * 128:(k + 1) * 128, :])
        for k in range(4):
            nc.scalar.dma_start(fw3_sb[:, k * 32:(k + 1) * 32],
                                fw3_d[k * 128:(k + 1) * 128, :])
        eye_sb = wpool.tile([128, 128], BF16)
        nc.scalar.dma_start(eye_sb[:, :], eye_d[:, :])
        # l3 output biases broadcast to 32 partitions: cols (pb3, fb3_0, fb3_1)
        b3s_sb = wpool.tile([1, 3], F32)
        nc.scalar.dma_start(b3s_sb[:, :], b3s_d[:, :])
        b3bc = wpool.tile([32, 3], F32)
        nc.gpsimd.partition_broadcast(b3bc[:, :], b3s_sb[:, :])
        ones3 = wpool.tile([32, NBLK * 3], F32)
        v.memset(ones3[:, :], 1.0)

        # PE warm-up: ~100 tiny matmuls on a zeroed tile during the DMA
        # preamble so phase 1 starts at the full 2.4 GHz HAM clock
        warm = wpool.tile([128, 128], BF16)
        v.memset(warm[:, :], 0.0)
        pwarm = psp.tile([128, NB], F32, tag="ps_main")
        for _ in range(56):
            te.matmul(pwarm[:, 0:128], warm[:, :], warm[:, :],
                      start=True, stop=True)

        # ---- phase 1: all macro-tile bases (PE-dense, keeps HAM warm) ---
        base_p, base_f, sts, raw9 = {}, {}, {}, {}
        pw13 = pw1_sb[:, :].rearrange("p (k m) -> p k m", k=4)
        fw13 = fw1_sb[:, :].rearrange("p (k m) -> p k m", k=4)
        for mt in range(NM):
            rows = slice(mt * NB, (mt + 1) * NB)
            seqT = stp.tile([128, 4 * NB], F8, tag="seqT", name=f"seqT{mt}")
            sq3 = seqT[:, :].rearrange("p (k n) -> p k n", k=4)
            nc.sync.dma_start(
                sq3[:, :, :],
                seq_d[:, rows].rearrange("(k p) n -> p k n", k=4))
            base_p[mt] = pers.tile([128, 4 * NB], BF16, tag=f"base_p{mt}", name=f"base_p{mt}")
            base_f[mt] = pers.tile([128, 4 * NB], BF16, tag=f"base_f{mt}", name=f"base_f{mt}")
            for m in range(4):
                pp = psp.tile([128, NB], F32, tag="ps_main")
                for j in range(2):
                    te.matmul(pp[:, :],
                              pw13[:, 2 * j:2 * j + 2, m * 128:(m + 1) * 128],
                              sq3[:, 2 * j:2 * j + 2, :],
                              start=(j == 0), stop=(j == 1),
                              perf_mode=DRMODE)
                v.tensor_scalar(base_p[mt][:, m * NB:(m + 1) * NB], pp[:, :],
                                ISC8, pb1_sb[:, m:m + 1], ALU.mult, ALU.add)
                pf_ = psp.tile([128, NB], F32, tag="ps_main")
                for j in range(2):
                    te.matmul(pf_[:, :],
                              fw13[:, 2 * j:2 * j + 2, m * 128:(m + 1) * 128],
                              sq3[:, 2 * j:2 * j + 2, :],
                              start=(j == 0), stop=(j == 1),
                              perf_mode=DRMODE)
                v.tensor_scalar(base_f[mt][:, m * NB:(m + 1) * NB], pf_[:, :],
                                ISC8, fb1_sb[:, m:m + 1], ALU.mult, ALU.add)
            sts[mt] = pers.tile([32, NBLK * 32], F32, tag=f"st{mt}", name=f"st{mt}")
            v.memset(sts[mt][:, :], 0.0)
            raw9[mt] = pers.tile([32, NBLK * 9], F32, tag=f"r9{mt}",
                                 name=f"r9{mt}")

        out9s = {}
        # ---- phase 2: autoregressive steps, macro-interleaved -----------
        # While macro mt's plumbing runs on DVE/ACT, the PE executes the
        # other macros' matmuls (engines are in-order; interleaved emission
        # is what lets the scheduler fill the gaps).
        for s in range(3):
            for mt in range(NM):
                st3 = r3(sts[mt], 32)
                gt_sb = smp.tile([32, NBLK * 3], F32, tag="gt")
                nc.sync.dma_start(
                    gt_sb[:, :],
                    gts_d[s * 32:(s + 1) * 32,
                          mt * NBLK * 3:(mt + 1) * NBLK * 3])
                gt3 = r3(gt_sb, 3)
                mk_sb = smp.tile([32, NBLK * 3], U8, tag="mi")
                nc.sync.dma_start(
                    mk_sb[:, :],
                    mski_d[s * 32:(s + 1) * 32,
                           mt * NBLK * 3:(mt + 1) * NBLK * 3])
                mk3 = r3(mk_sb, 3)
                roi_sb = smp.tile([32, NBLK * 9], U8, tag="roi")
                nc.sync.dma_start(
                    roi_sb[:, :],
                    roi9_d[s * 32:(s + 1) * 32,
                           mt * NBLK * 9:(mt + 1) * NBLK * 9])
                roi9 = r3(roi_sb, 9)
                roh_sb = smp.tile([32, NBLK * 3], F32, tag="rohs")
                nc.sync.dma_start(
                    roh_sb[:, :],
                    roh_d[s * 32:(s + 1) * 32,
                          mt * NBLK * 3:(mt + 1) * NBLK * 3])
                roh3 = r3(roh_sb, 3)

                # bridge to feature-major: cast + stream-transpose
                # (st slots 6..8 stay zero; the one-hot joins here)
                st_bf = smp.tile([32, NBLK * 32], BF16, tag="stbf")
                v.tensor_copy(st_bf[:, :], sts[mt][:, :])
                v.tensor_copy(r3(st_bf, 32)[:, :, S_ROH:S_ROH + 3],
                              roh3[:, :, :])  # slots 12:15
                exT = smp.tile([128, NBLK * 32], BF16, tag="exT")
                v.transpose(exT[0:32, :], st_bf[:, :])
                v.tensor_copy(exT[32:47, :], exT[0:15, :])
                v.tensor_copy(exT[64:79, :], exT[0:15, :])
                v.tensor_copy(exT[96:111, :], exT[0:15, :])
                if s == 2:
                    # final step: outputs assemble in out9 (pre-scatter copy
                    # now, scatter lands directly in it; st goes stale)
                    out9s[mt] = smp.tile([32, NBLK * 9], F32, tag="out9",
                                         name=f"out9_{mt}")
                    v.tensor_copy(r3(out9s[mt], 9)[:, :, :], st3[:, :, 0:9])

                # layer 1: base + extra @ w1x -> gelu -> h1 (bf16)
                h1p = bigp.tile([128, 4 * NB], F8, tag="h1p")
                h1f = bigp.tile([128, 4 * NB], F8, tag="h1f")
                for half in range(2):
                    quad = []
                    for i in range(2):
                        m = 2 * half + i
                        pp = psp.tile([128, NB], F32, tag="ps_main")
                        pf_ = psp.tile([128, NB], F32, tag="ps_main")
                        te.matmul(pp[:, :],
                                  w1xq_sb[32 * i:32 * i + 15,
                                          m * 128:(m + 1) * 128],
                                  exT[32 * i:32 * i + 15, :],
                                  start=True, stop=False,
                                  tile_position=(32 * i, 0))
                        te.matmul(pf_[:, :],
                                  w1xq_sb[64 + 32 * i:64 + 32 * i + 15,
                                          m * 128:(m + 1) * 128],
                                  exT[64 + 32 * i:64 + 32 * i + 15, :],
                                  start=True, stop=False,
                                  tile_position=(64 + 32 * i, 0))
                        quad.append((m, pp, pf_))
                    for m, pp, pf_ in quad:
                        te.matmul(pp[:, :], eye_sb[:, :],
                                  base_p[mt][:, m * NB:(m + 1) * NB],
                                  start=False, stop=True)
                        te.matmul(pf_[:, :], eye_sb[:, :],
                                  base_f[mt][:, m * NB:(m + 1) * NB],
                                  start=False, stop=True)
                        sc.activation(h1p[:, m * NB:(m + 1) * NB], pp[:, :],
                                      AF.Gelu)
                        sc.activation(h1f[:, m * NB:(m + 1) * NB], pf_[:, :],
                                      AF.Gelu)

                # layer 2
                h2p = bigp.tile([128, 2 * NB], F8, tag="h2p")
                h1p3 = h1p[:, :].rearrange("p (k n) -> p k n", k=4)
                pw23 = pw2_sb[:, :].rearrange("p (k m) -> p k m", k=4)
                for m in range(2):
                    pp = psp.tile([128, NB], F32, tag="ps_main")
                    for j in range(2):
                        te.matmul(pp[:, :],
                                  pw23[:, 2 * j:2 * j + 2,
                                       m * 128:(m + 1) * 128],
                                  h1p3[:, 2 * j:2 * j + 2, :],
                                  start=(j == 0), stop=(j == 1),
                                  perf_mode=DRMODE)
                    sc.activation(h2p[:, m * NB:(m + 1) * NB], pp[:, :],
                                  AF.Gelu, scale=ISC8, bias=pb2_sb[:, m:m + 1])
                h2f = bigp.tile([128, 4 * NB], F8, tag="h2f")
                h1f3 = h1f[:, :].rearrange("p (k n) -> p k n", k=4)
                fw23 = fw2_sb[:, :].rearrange("p (k m) -> p k m", k=4)
                for m in range(4):
                    pp = psp.tile([128, NB], F32, tag="ps_main")
                    for j in range(2):
                        te.matmul(pp[:, :],
                                  fw23[:, 2 * j:2 * j + 2,
                                       m * 128:(m + 1) * 128],
                                  h1f3[:, 2 * j:2 * j + 2, :],
                                  start=(j == 0), stop=(j == 1),
                                  perf_mode=DRMODE)
                    sc.activation(h2f[:, m * NB:(m + 1) * NB], pp[:, :],
                                  AF.Gelu, scale=ISC8, bias=fb2_sb[:, m:m + 1])

                # layer 3 (padded to M=32; pres in col-group 0, fe in
                # col-group 1 of the same psum tile -> MMs run concurrently)
                # L3 merged: logit in M-col 0, pf/pe in M-cols 1:3 of one
                # zero-padded [32, NB] psum accumulation group
                p3 = ps3.tile([32, NB], F32, tag="ps3")
                pw33 = pw3_sb[:, :].rearrange("p (k m) -> p k m", k=2)
                fw33 = fw3_sb[:, :].rearrange("p (k m) -> p k m", k=4)
                h2p3 = h2p[:, :].rearrange("p (k n) -> p k n", k=2)
                h2f3 = h2f[:, :].rearrange("p (k n) -> p k n", k=4)
                te.matmul(p3[:, :], pw33[:, 0:2, :], h2p3[:, 0:2, :],
                          start=True, stop=False, perf_mode=DRMODE)
                for j in range(2):
                    te.matmul(p3[:, :], fw33[:, 2 * j:2 * j + 2, :],
                              h2f3[:, 2 * j:2 * j + 2, :],
                              start=False, stop=(j == 1), perf_mode=DRMODE)

                # bridge back to blocked batch-major; add b3 biases after
                lgT = smp.tile([32, NBLK * 32], F32, tag="lgT")
                v.transpose(lgT[:, :], p3[:, :])
                lg3 = r3(lgT, 32)
                fe3 = lg3
                r9v = r3(raw9[mt], 9)
                logit = r9v[:, :, s:s + 1]
                pf = r9v[:, :, 3 + s:4 + s]
                pe = r9v[:, :, 6 + s:7 + s]
                v.tensor_scalar(logit, lg3[:, :, 0:1], ISC8, b3bc[:, 0:1],
                                ALU.mult, ALU.add)
                v.tensor_scalar(pf, fe3[:, :, 1:2], ISC8, b3bc[:, 1:2],
                                ALU.mult, ALU.add)
                v.tensor_scalar(pe, fe3[:, :, 2:3], ISC8, b3bc[:, 2:3],
                                ALU.mult, ALU.add)

                # ---- plumbing: acts slots 0:3 (p,f,e), preds 3:6 --------
                pb = smp.tile([32, NBLK * 8], F32, tag="pb")
                pb3d = r3(pb, 8)
                acts = pb3d[:, :, 0:3]
                sig, pfc, pec = pb3d[:, :, 3:4], pb3d[:, :, 4:5], pb3d[:, :, 5:6]

                # sigmoid(l) = 0.5*tanh(0.5*l) + 0.5 (stays on the gelu table)
                sc.activation(sig, logit, AF.Tanh, scale=0.5)
                v.tensor_scalar(sig, sig, 0.5, 0.5, ALU.mult, ALU.add)
                v.tensor_scalar(pfc, pf, -10.0, 10.0, ALU.max, ALU.min)
                v.tensor_scalar(pec, pe, -100.0, 100.0, ALU.max, ALU.min)
                v.tensor_copy(acts, gt3[:, :, :])
                v.copy_predicated(acts, mk3[:, :, :], pb3d[:, :, 3:6])

                # state scatter: one 3-wide predicated copy per round, flags
                # via ones under a stride-3 view of the 9-wide round one-hot
                sdst = r3(out9s[mt], 9) if s == 2 else st3
                for r in range(3):
                    v.copy_predicated(sdst[:, :, 3 * r:3 * r + 3],
                                      roi9[:, :, 3 * r:3 * r + 3], acts)
                if s < 2:
                    v.copy_predicated(st3[:, :, S_FL:S_FL + 3],
                                      roi9[:, :, 0:9:3],
                                      r3(ones3, 3)[:, :, :])

                if s == 2:
                    rows = slice(mt * NB, (mt + 1) * NB)
                    nc.sync.dma_start(
                        dpfe_d[rows, :].rearrange("(j p) r -> p j r", p=32),
                        r3(out9s[mt], 9)[:, :, :])
                    nc.gpsimd.dma_start(
                        raw9_d[rows, :].rearrange("(j p) r -> p j r", p=32),
                        r9v[:, :, :])

        for p in (ps3, psp, smp, stp, bigp, pers, wpool):
            p.release()

    nc.compile()
    return nc


# ---------------------------------------------------------------------------
def prep_inputs(seq_embed, freq, pres, enrich,
                pw1, pb1, pw2, pb2, pw3, pb3,
                fw1, fb1, fw2, fb2, fw3, fb3,
                perm_idx, round_mask, BL):
    """Host-side (numpy) sharding + index preprocessing."""
    f32 = np.float32
    seq = np.asarray(seq_embed, f32)
    perms = ALL_PERMS[np.asarray(perm_idx)]                    # [B,3]
    gtf = np.take_along_axis(np.asarray(freq, f32), perms, 1)   # [B,3] (col=s)
    gtp = np.take_along_axis(np.asarray(pres, f32), perms, 1)
    gte = np.take_along_axis(np.asarray(enrich, f32), perms, 1)
    m = np.take_along_axis(np.asarray(round_mask), perms, 1).astype(f32)
    roh = (perms[:, :, None] == np.arange(3)[None, None, :]).astype(f32)  # [B,3s,3r]

    bf = lambda a: np.ascontiguousarray(np.asarray(a, f32).astype(NP_BF16))
    f8s = lambda a: np.ascontiguousarray(
        (np.asarray(a, f32) * SC8).astype(NP_F8))
    # extras rows in r-major slot order (p,f,e per round, flags, roh)
    pw1a = np.asarray(pw1, f32); fw1a = np.asarray(fw1, f32)
    pw1x = np.zeros((15, H), f32)
    for r in range(3):
        pw1x[3 * r] = pw1a[512 + 2 * r]          # pres_r
        pw1x[9 + r] = pw1a[513 + 2 * r]          # flag_r
        pw1x[12 + r] = pw1a[518 + r]             # roh_r
    fw1x = np.zeros((15, H), f32)
    for r in range(3):
        fw1x[3 * r + 0] = fw1a[513 + 4 * r]      # pres_r
        fw1x[3 * r + 1] = fw1a[512 + 4 * r]      # freq_r
        fw1x[3 * r + 2] = fw1a[514 + 4 * r]      # enrich_r
        fw1x[9 + r] = fw1a[515 + 4 * r]          # flag_r
        fw1x[12 + r] = fw1a[524 + r]             # roh_r
    w1xq = np.zeros((128, H), f32)
    w1xq[0:15] = pw1x; w1xq[32:47] = pw1x
    w1xq[64:79] = fw1x; w1xq[96:111] = fw1x
    w1xq = bf(w1xq)
    pw3p = np.zeros((256, 32), f32); pw3p[:, 0] = np.asarray(pw3, f32)[:, 0]
    fw3p = np.zeros((512, 32), f32); fw3p[:, 1:3] = np.asarray(fw3, f32)
    b3s = np.array([[np.asarray(pb3, f32)[0],
                     np.asarray(fb3, f32)[0], np.asarray(fb3, f32)[1]]], f32)

    shared = {
        "pw1": f8s(np.asarray(pw1, f32)[:512]), "w1xq": w1xq,
        "pb1": np.ascontiguousarray(np.asarray(pb1, f32)),
        "pw2": np.ascontiguousarray((np.asarray(pw2, f32) * SC8).astype(NP_F8)),
        "pb2": np.ascontiguousarray(np.asarray(pb2, f32)),
        "pw3p": f8s(pw3p),
        "fw1": f8s(np.asarray(fw1, f32)[:512]),
        "fb1": np.ascontiguousarray(np.asarray(fb1, f32)),
        "fw2": np.ascontiguousarray((np.asarray(fw2, f32) * SC8).astype(NP_F8)),
        "fb2": np.ascontiguousarray(np.asarray(fb2, f32)),
        "fw3p": f8s(fw3p),
        "eye": np.eye(128, dtype=NP_BF16),
        "b3s": b3s,
    }

    in_maps = []
    ncores = seq.shape[0] // BL
    BLKT = BL // 32
    for c in range(ncores):
        rs = slice(c * BL, (c + 1) * BL)
        # blocked layouts: index [s*32+p, Jg*w + q], b_local = 32*Jg + p
        gt3 = np.stack([gtp[rs], gtf[rs], gte[rs]], -1)          # [BL,3s,3]
        gt3 = gt3.reshape(BLKT, 32, 3, 3).transpose(2, 1, 0, 3)  # [3s,32,J,3]
        rohc = roh[rs].reshape(BLKT, 32, 3, 3).transpose(2, 1, 0, 3)
        mk3 = np.repeat(m[rs][:, :, None], 3, axis=2)
        mk3 = mk3.reshape(BLKT, 32, 3, 3).transpose(2, 1, 0, 3)
        r9 = np.repeat(roh[rs], 3, axis=2)                       # [BL,3s,9]
        r9 = r9.reshape(BLKT, 32, 3, 9).transpose(2, 1, 0, 3)
        in_maps.append(dict(
            seq=np.ascontiguousarray(seq[rs].astype(NP_F8).T),
            gts=np.ascontiguousarray(gt3.reshape(3 * 32, BLKT * 3)),
            roh=np.ascontiguousarray(rohc.reshape(3 * 32, BLKT * 3)),
            mski=np.ascontiguousarray(
                mk3.reshape(3 * 32, BLKT * 3).astype(np.uint8)),
            roi9=np.ascontiguousarray(
                r9.reshape(3 * 32, BLKT * 9).astype(np.uint8)),
            **shared))
    aux = dict(gtf=gtf, gtp=gtp, gte=gte, m=m)
    return in_maps, aux


def assemble(results, aux):
    """Gather per-core outputs; finish the (tiny) loss reductions host-side."""
    f32 = np.float32
    dpfe = np.concatenate([r["dpfe"] for r in results], 0).astype(f32)
    dpfe = dpfe.reshape(-1, 3, 3)
    dp, df, de = dpfe[:, :, 0], dpfe[:, :, 1], dpfe[:, :, 2]
    r9 = np.concatenate([r["raw9"] for r in results], 0).astype(f32)
    lg, pf, pe = r9[:, 0:3], r9[:, 3:6], r9[:, 6:9]
    m, gtf, gtp, gte = aux["m"], aux["gtf"], aux["gtp"], aux["gte"]
    lf = np.sum(np.square(pf - gtf) * m, dtype=np.float64)
    le = np.sum(np.square(pe - gte) * m, dtype=np.float64)
    bce = (np.maximum(lg, 0.0) - lg * gtp
           + np.log1p(np.exp(-np.abs(lg), dtype=np.float64)))
    lp = np.sum(bce * m, dtype=np.float64)
    nm = np.sum(m, dtype=np.float64) + 1e-8
    head = np.array([lf / nm, lp / nm, le / nm], f32)
    return np.concatenate([head, df.ravel(), dp.ravel(), de.ravel()])


_CACHE = {}


def _get_graph(BL):
    if BL not in _CACHE:
        _CACHE[BL] = build_graph(BL)
    return _CACHE[BL]


def _install_profile_hook():
    """Provide antenv.axon_hooks (missing in this image) so trace=True works."""
    import sys, types
    try:
        import antenv.axon_hooks  # noqa: F401
        return
    except ImportError:
        pass
    from trn_agent_boot.trn_boot import _ntff_profile_via_ctypes
    hook = _ntff_profile_via_ctypes('/opt/axon/libaxon_pjrt.so')
    mod = types.ModuleType('antenv.axon_hooks')
    mod._hook = hook
    mod.get_axon_ntff_profile_hook = lambda: mod._hook
    mod.set_axon_ntff_profile_hook = lambda h: setattr(mod, '_hook', h)
    sys.modules['antenv.axon_hooks'] = mod


def run(inputs, trace=False):
    if trace:
        _install_profile_hook()
    BL = inputs["seq_embed"].shape[0] // NCORES
    nc = _get_graph(BL)
    in_maps, aux = prep_inputs(**inputs, BL=BL)
    res = run_bass_kernel_spmd(nc, in_maps, core_ids=list(range(NCORES)),
                               trace=trace)
    out = assemble(res.results, aux)
    return out, res


def kernel(**inputs):
    inputs = {k: np.asarray(v) for k, v in inputs.items()}
    out, _ = run(inputs)
    return out

